# revision 25
# baseline (speedup 1.0000x reference)
"""Trainium2 Bass kernel for fused LayerNorm + causal multi-head attention.

Reference computation (B=2, S=2048, M=2048, H=16, D=128):
    norm = layernorm(x) * ln_w + ln_b
    qkv  = norm @ qkvw.T + qkvb            -> q, k, v  (B,S,H,D)
    out  = softmax_causal(q k^T / sqrt(D)) v @ ow.T + ob

Sharding across 8 NeuronCores (tensor parallel, heads 2/core):
    - Host pre-transposes/pre-tiles x and the weights so every bulk DMA is
      large contiguous lines (16KB+ per partition; DMA engines here are
      per-packet latency-bound, so packet size is the throughput lever);
      LayerNorm affine is folded into the QKV weights and the
      standardization applied algebraically AFTER the QKV matmul.
    - NO DRAM round-trips for partition broadcasts: LayerNorm stats are
      transposed on the PE (identity matmul) and replicated across
      partitions with K=1 ones-row matmuls into PSUM; the same trick
      broadcasts softmax denominators and the output bias.
    - Column-parallel QKV producing q^T/k^T (head-dim-major) and v
      (seq-major) in per-512-column tiles; LayerNorm stats chains emitted
      one chunk ahead of use in groups of 4 row tiles.
    - Attention head-major ACROSS batches; causal 0/1 mask multiply on
      exp() of diagonal tiles; softmax denominators accumulated on DVE and
      partition-reduced with a single ones-matmul per query chunk. The
      scalar engine runs ONLY exp in the attention phase (no activation-
      table thrash); PSUM evacuation happens on DVE.
    - TWO fp16 AllToAlls (one per local head) flip head-sharding ->
      sequence-sharding: A2A#0 fires at the attention midpoint so its
      output gather + normalization hide behind the second half of
      attention; the output projection contracts A2A#0's heads first so
      those matmuls run while A2A#1 flies.
    - Row-local output projection (full ow, prefetched during attention
      into the SBUF slot the QKV weights vacate); outputs staged into
      full 2048-column rows so the final writes are 8KB-line DMAs.

DMA queue assignment (a collective blocks its issuing engine's queue until
completion, so gpsimd carries NOTHING but collectives):
    - nc.gpsimd: collectives ONLY
    - nc.sync:   bulk streaming (xT chunks, x row groups, weights, output)
    - nc.scalar: a2a staging writes, small constants
"""

import sys
import types

import numpy as np

B = 2
S = 2048
M = 2048
H = 16
D = 128
EPS = 1e-5
NCORES = 8
ROWS = B * S                  # 4096 flattened sequence rows
SHARD = ROWS // NCORES        # 512 rows per core
HPC = H // NCORES             # 2 heads per core
NQK = 2 * HPC * D             # 512 q+k features per core
NV = HPC * D                  # 256 v features per core
NW = NQK + NV                 # 768 qkv features per core
CHUNK = 512                   # QKV pipeline sequence chunk width
QCHUNK = 512                  # attention query chunk width
MCHUNK = 512                  # output projection feature chunk
MT = M // 128                 # 16
RT = S // 128                 # 16 row tiles per batch
QC = S // QCHUNK              # 4 query chunks per batch
NCH = S // CHUNK              # 4 QKV chunks per batch


def _install_ntff_hook():
    """Register the axon NTFF profiling hook if available (timing only)."""
    if "antenv.axon_hooks" in sys.modules:
        return
    mod = types.ModuleType("antenv.axon_hooks")
    _h = [None]
    mod.set_axon_ntff_profile_hook = lambda h: _h.__setitem__(0, h)
    mod.get_axon_ntff_profile_hook = lambda: _h[0]
    sys.modules["antenv.axon_hooks"] = mod
    try:
        import antenv

        antenv.axon_hooks = mod
    except ImportError:
        pass
    try:
        from trn_agent_boot.trn_boot import _ntff_profile_via_ctypes

        hook = _ntff_profile_via_ctypes("/opt/axon/libaxon_pjrt.so")
        if hook is not None:
            mod.set_axon_ntff_profile_hook(hook)
    except Exception:
        pass


_NC_CACHE = {}


def _build_program():
    import concourse.bass as bass
    import concourse.mybir as mybir
    import concourse.tile as tile
    from concourse import bacc

    f32 = mybir.dt.float32
    f16 = mybir.dt.float16
    AFT = mybir.ActivationFunctionType
    ALU = mybir.AluOpType

    nc = bacc.Bacc("TRN2", target_bir_lowering=False, debug=False,
                   num_devices=NCORES)

    # ---- kernel I/O -----------------------------------------------------
    # natural x in groups of 2 row tiles: xng[g, p, j, m] =
    #   x[g*256 + j*128 + p, m]  (8KB contiguous per (g, p))
    xng_in = nc.dram_tensor("xng", [2 * B * NCH, 128, 2, M], f16,
                            kind="ExternalInput")
    # pre-tiled transposed x: xtp[c, p, mt, s] = x[c*512 + s, mt*128 + p]
    xtp_in = nc.dram_tensor("xtp", [B * NCH, 128, MT, CHUNK], f16,
                            kind="ExternalInput")
    # pre-tiled qkv weights: wtp[p, mt, n] = W'[n, mt*128 + p]
    wtp_in = nc.dram_tensor("wtp", [128, MT, NW], f16, kind="ExternalInput")
    wsqk_in = nc.dram_tensor("wsum_qk", [NQK], f32, kind="ExternalInput")
    wsv_in = nc.dram_tensor("wsum_v", [NV], f32, kind="ExternalInput")
    bqk_in = nc.dram_tensor("bqk", [NQK], f32, kind="ExternalInput")
    bv_in = nc.dram_tensor("bv", [NV], f32, kind="ExternalInput")
    # pre-tiled out-proj weights: owtp[p, t, n] = ow[n, t*128 + p]
    owtp_in = nc.dram_tensor("owtp", [128, MT, M], f16, kind="ExternalInput")
    ob_in = nc.dram_tensor("ob", [M], f32, kind="ExternalInput")
    mask_in = nc.dram_tensor("mask_const", [4, 128, QCHUNK], f16,
                             kind="ExternalInput")
    ident_in = nc.dram_tensor("ident", [128, 128], f16, kind="ExternalInput")
    out_ext = nc.dram_tensor("out_shard", [SHARD, M], f32,
                             kind="ExternalOutput")

    # ---- internal DRAM --------------------------------------------------
    wa2a_in = nc.dram_tensor("wa2a_in", [NCORES, 8, 512], f16)
    wa2a_out = nc.dram_tensor("wa2a_out", [NCORES, 8, 512], f16)
    # half-A2A K ships local head K's NORMALIZED ctx (128 rows)
    a2a_in = [nc.dram_tensor(f"a2a_in{k}", [NCORES, 128, SHARD], f16)
              for k in range(HPC)]
    a2a_out = [nc.dram_tensor(f"a2a_out{k}", [NCORES, 128, SHARD], f16)
               for k in range(HPC)]

    rg = [list(range(NCORES))]

    with tile.TileContext(nc) as tc:
        # small warm-up A2A: absorbs ncfw/algorithm first-call setup
        # concurrently with the QKV phase (gpsimd carries only collectives,
        # so nothing queues behind it)
        nc.gpsimd.collective_compute(
            "AllToAll", mybir.AluOpType.bypass,
            replica_groups=rg,
            ins=[wa2a_in.ap().opt()],
            outs=[wa2a_out.ap().opt()],
        )

        with tc.tile_pool(name="persist", bufs=1) as persist, \
             tc.tile_pool(name="stat_sb", bufs=1) as stp, \
             tc.tile_pool(name="ps", bufs=1, space="PSUM") as psp, \
             tc.tile_pool(name="wts", bufs=1) as wtp:
            # persistent SBUF constants
            eps_t = persist.tile([128, 1], f32, tag="eps")
            nc.vector.memset(eps_t, EPS)
            ones_t = persist.tile([128, 1], f16, tag="ones")
            nc.vector.memset(ones_t, 1.0)
            onesr_t = persist.tile([1, 128], f16, tag="onesr")
            nc.vector.memset(onesr_t, 1.0)
            onesr32_t = persist.tile([1, 128], f32, tag="onesr32")
            nc.vector.memset(onesr32_t, 1.0)
            ident_t = persist.tile([128, 128], f16, tag="ident")
            nc.scalar.dma_start(ident_t[:], ident_in.ap())
            bqk_t = persist.tile([128, 4], f32, tag="bqk")
            nc.scalar.dma_start(bqk_t[:],
                                bqk_in.ap().rearrange("(n p) -> p n", p=128))
            wsqk_t = persist.tile([128, 4], f32, tag="wsqk")
            nc.scalar.dma_start(
                wsqk_t[:], wsqk_in.ap().rearrange("(n p) -> p n", p=128))
            bv_t = persist.tile([128, NV], f32, tag="bv")
            nc.scalar.dma_start(
                bv_t[:],
                bass.AP(tensor=bv_in, offset=0, ap=[[0, 128], [1, NV]]))
            wsv_t = persist.tile([128, NV], f32, tag="wsv")
            nc.scalar.dma_start(
                wsv_t[:],
                bass.AP(tensor=wsv_in, offset=0, ap=[[0, 128], [1, NV]]))
            # 4 causal 0/1 mask tiles in scores^T layout [k_part, q_free]:
            # mask_t[i, j] = 1.0 iff (128*t + i) <= j
            masks = []
            for t in range(4):
                mt_ = persist.tile([128, QCHUNK], f16, tag=f"mask{t}",
                                   name=f"mask{t}")
                nc.scalar.dma_start(mt_[:], mask_in[t, :, :])
                masks.append(mt_)

            # per-batch natural-orientation stats kept in SBUF for v-path
            rstd_all = [stp.tile([128, RT], f32, tag=f"rstd{b}",
                                 name=f"rstd{b}") for b in range(B)]
            rm_all = [stp.tile([128, RT], f32, tag=f"rm{b}",
                               name=f"rm{b}") for b in range(B)]

            # qkv weights and (later) out-proj weights share ONE slot: the
            # ow load starts automatically once the last QKV matmul is done
            # (loaded in two halves so the first matmuls start sooner)
            wt_sb = wtp.tile([128, MT, NW], f16, tag="wslot")
            nc.sync.dma_start(wt_sb[:, 0:MT // 2, :],
                              wtp_in.ap()[:, 0:MT // 2, :])

            with tc.tile_pool(name="xs", bufs=2) as xsp, \
                 tc.tile_pool(name="lnsmall", bufs=4) as lns, \
                 tc.tile_pool(name="nstream", bufs=2) as nsp, \
                 tc.tile_pool(name="rstream", bufs=2) as rsp, \
                 tc.tile_pool(name="qkv", bufs=1) as qkvp, \
                 tc.tile_pool(name="attn", bufs=3) as atp, \
                 tc.tile_pool(name="ctxp", bufs=2) as ctp:

                def stats_group(g):
                    """Stats for global row tiles 2g..2g+1, fed by ONE
                    2-tile x load (8KB lines on the sync queue)."""
                    b, r0 = g // (2 * NCH), 2 * (g % (2 * NCH))
                    x_t = xsp.tile([128, 2, M], f16, tag="x_t", name="x_t")
                    nc.sync.dma_start(x_t[:], xng_in[g, :, :, :])
                    mvg = lns.tile([128, 2, 2], f32, tag="mvg", name="mvg")
                    for j in range(2):
                        stats = lns.tile([128, 4, 6], f32, tag="stats",
                                         name="stats")
                        xg = x_t[:, j, :].rearrange("p (g d) -> p g d", g=4)
                        for gg in range(4):
                            nc.vector.bn_stats(out=stats[:, gg, :],
                                               in_=xg[:, gg, :])
                        nc.vector.bn_aggr(out=mvg[:, j, :], in_=stats[:])
                    c0, c1 = r0, r0 + 2
                    sd = lns.tile([128, 2], f32, tag="sd", name="sd")
                    nc.scalar.activation(out=sd[:], in_=mvg[:, :, 1],
                                         func=AFT.Sqrt, bias=eps_t[:],
                                         scale=1.0)
                    nc.vector.reciprocal(out=rstd_all[b][:, c0:c1],
                                         in_=sd[:])
                    nc.vector.tensor_tensor(
                        out=rm_all[b][:, c0:c1], in0=mvg[:, :, 0],
                        in1=rstd_all[b][:, c0:c1], op=ALU.mult)

                # group emission schedule: one chunk ahead of consumers
                # (global chunk c consumes groups 2c, 2c+1); chunk 0's xT
                # load is hoisted ahead of the prologue stats groups so the
                # first matmuls are not queued behind them
                ahead = {c: (2 * c + 2, 2 * c + 3) for c in range(7)}
                ahead[7] = ()
                xt0 = nsp.tile([128, MT, CHUNK], f16, tag="xt_t",
                               name="xt_t")
                nc.sync.dma_start(xt0[:], xtp_in[0, :, :, :])
                nc.sync.dma_start(wt_sb[:, MT // 2:MT, :],
                                  wtp_in.ap()[:, MT // 2:MT, :])
                stats_group(0)
                stats_group(1)

                qkT = [[[qkvp.tile([128, QCHUNK], f16,
                                   tag=f"qkT{b}_{i}_{q}",
                                   name=f"qkT{b}_{i}_{q}")
                         for q in range(QC)] for i in range(4)]
                       for b in range(B)]
                vN = [[qkvp.tile([128, 4, NV], f16, tag=f"vN{b}_{q}",
                                 name=f"vN{b}_{q}") for q in range(QC)]
                      for b in range(B)]

                # --- QKV pipeline over 512-column sequence chunks --------
                for b in range(B):
                    for chb in range(NCH):
                        if b == 0 and chb == 0:
                            xt_t = xt0
                        else:
                            xt_t = nsp.tile([128, MT, CHUNK], f16,
                                            tag="xt_t", name="xt_t")
                            nc.sync.dma_start(
                                xt_t[:], xtp_in[b * NCH + chb, :, :, :])
                        for g in ahead[b * NCH + chb]:
                            stats_group(g)
                        # partition-broadcast rstd/rm for this chunk:
                        # PE transpose -> [4, 128] rows -> K=1 ones-row
                        # matmul outer product -> [128, 512] in PSUM -> SBUF
                        c0, c1 = 4 * chb, 4 * chb + 4
                        s16 = rsp.tile([128, 2, 4], f16, tag="s16",
                                       name="s16")
                        nc.vector.tensor_copy(out=s16[:, 0, :],
                                              in_=rstd_all[b][:, c0:c1])
                        nc.vector.tensor_copy(out=s16[:, 1, :],
                                              in_=rm_all[b][:, c0:c1])
                        trT = psp.tile([4, 128], f16, tag="t3",
                                       name="trT", bufs=2)
                        nc.tensor.transpose(trT[:], s16[:, 0, :],
                                            ident_t[:])
                        trR = psp.tile([4, 128], f16, tag="t3",
                                       name="trR", bufs=2)
                        nc.tensor.transpose(trR[:], s16[:, 1, :],
                                            ident_t[:])
                        stT = rsp.tile([4, 128], f16, tag="stT", name="stT")
                        nc.vector.tensor_copy(out=stT[:], in_=trT[:])
                        stR = rsp.tile([4, 128], f16, tag="stR", name="stR")
                        nc.vector.tensor_copy(out=stR[:], in_=trR[:])
                        # gather the 4 rows into one partition-0 row (tiny
                        # on-chip DMA) so the broadcast matmul rhs is
                        # 32-aligned, then one N=512 outer-product matmul
                        srT = rsp.tile([1, CHUNK], f16, tag="srT",
                                       name="srT")
                        nc.scalar.dma_start(srT[:], stT[:])
                        srR = rsp.tile([1, CHUNK], f16, tag="srR",
                                       name="srR")
                        nc.scalar.dma_start(srR[:], stR[:])
                        rps = psp.tile([128, CHUNK], f32, tag="acc2",
                                       name="rps", bufs=2)
                        rmps = psp.tile([128, CHUNK], f32, tag="acc2",
                                        name="rmps", bufs=2)
                        nc.tensor.matmul(rps[:], onesr_t[:], srT[:],
                                         start=True, stop=True)
                        nc.tensor.matmul(rmps[:], onesr_t[:], srR[:],
                                         start=True, stop=True)
                        r_b = rsp.tile([128, CHUNK], f16, tag="r_b",
                                       name="r_b")
                        nc.vector.tensor_copy(out=r_b[:], in_=rps[:])
                        rm_b = rsp.tile([128, CHUNK], f16, tag="rm_b",
                                        name="rm_b")
                        nc.vector.tensor_copy(out=rm_b[:], in_=rmps[:])
                        # q/k features: out [n 128, s CHUNK]
                        for nt in range(4):
                            pqk = psp.tile([128, QCHUNK], f32, tag="acc1",
                                           name="pqk", bufs=2)
                            for mt in range(MT):
                                nc.tensor.matmul(
                                    pqk[:],
                                    wt_sb[:, mt, nt * 128:(nt + 1) * 128],
                                    xt_t[:, mt, :],
                                    start=(mt == 0), stop=(mt == MT - 1))
                            # qkT = raw*rstd[s] - (rm[s]*wsum[n] - c2[n])
                            t2 = rsp.tile([128, CHUNK], f16, tag="t2",
                                          name="t2")
                            nc.vector.tensor_scalar(
                                out=t2[:], in0=rm_b[:],
                                scalar1=wsqk_t[:, nt:nt + 1],
                                scalar2=bqk_t[:, nt:nt + 1],
                                op0=ALU.mult, op1=ALU.subtract)
                            traw = rsp.tile([128, CHUNK], f16, tag="traw",
                                            name="traw")
                            nc.vector.tensor_mul(out=traw[:],
                                                 in0=pqk[:],
                                                 in1=r_b[:])
                            nc.vector.tensor_tensor(
                                out=qkT[b][nt][chb][:],
                                in0=traw[:], in1=t2[:], op=ALU.subtract)
                        # v features: out [s 128, n 256]
                        for st in range(CHUNK // 128):
                            rt = chb * (CHUNK // 128) + st
                            pv = psp.tile([128, NV], f32, tag="acc2",
                                          name="pv", bufs=2)
                            for mt in range(MT):
                                nc.tensor.matmul(
                                    pv[:],
                                    xt_t[:, mt, st * 128:(st + 1) * 128],
                                    wt_sb[:, mt, NQK:NW],
                                    start=(mt == 0), stop=(mt == MT - 1))
                            # v = raw*rstd[s] - rm[s]*wsum_v[n] + bv[n]
                            tv = rsp.tile([128, NV], f16, tag="tv",
                                          name="tv")
                            nc.vector.tensor_scalar(
                                out=tv[:], in0=pv[:],
                                scalar1=rstd_all[b][:, rt:rt + 1],
                                scalar2=None, op0=ALU.mult)
                            t2v = rsp.tile([128, NV], f16, tag="t2v",
                                           name="t2v")
                            nc.vector.tensor_scalar(
                                out=t2v[:], in0=wsv_t[:],
                                scalar1=rm_all[b][:, rt:rt + 1],
                                scalar2=None, op0=ALU.mult)
                            t3v = rsp.tile([128, NV], f16, tag="t3v",
                                           name="t3v")
                            nc.vector.tensor_tensor(
                                out=t3v[:], in0=tv[:], in1=t2v[:],
                                op=ALU.subtract)
                            nc.vector.tensor_add(
                                out=vN[b][chb][:, st, :], in0=t3v[:],
                                in1=bv_t[:])

                # out-proj weight prefetch into the slot the QKV weights
                # occupy (dep auto-inserted via the shared tag): loads
                # during attention
                ow_sb = wtp.tile([128, MT, M], f16, tag="wslot")
                nc.sync.dma_start(ow_sb[:], owtp_in.ap())

                # --- attention, head-major across batches ----------------
                for hl in range(HPC):
                    for b in range(B):
                        ctxu, rds = [], []
                        for qc in range(QC):
                            pctx = psp.tile([128, QCHUNK], f32, tag="ctx",
                                            name="pctx", bufs=2)
                            exs = atp.tile([128, QCHUNK], f16, tag="exs",
                                           name="exs", bufs=2)
                            nkt = 4 * (qc + 1)
                            for kt in range(nkt):
                                ps_s = psp.tile([128, QCHUNK], f32,
                                                tag="t3", name="ps_s",
                                                bufs=2)
                                nc.tensor.matmul(
                                    ps_s[:],
                                    qkT[b][2 + hl][kt // 4]
                                    [:, (kt % 4) * 128:(kt % 4 + 1) * 128],
                                    qkT[b][hl][qc][:],
                                    start=True, stop=True)
                                ex = atp.tile([128, QCHUNK], f16, tag="ex",
                                              name="ex")
                                nc.scalar.activation(out=ex[:], in_=ps_s[:],
                                                     func=AFT.Exp,
                                                     scale=1.0)
                                if kt >= 4 * qc:
                                    nc.vector.tensor_mul(
                                        out=ex[:], in0=ex[:],
                                        in1=masks[kt - 4 * qc][:])
                                first, last = kt == 0, kt == nkt - 1
                                nc.tensor.matmul(
                                    pctx[:],
                                    vN[b][kt // 4][:, kt % 4,
                                                   hl * 128:(hl + 1) * 128],
                                    ex[:], start=first, stop=last)
                                # denominator accumulation on DVE
                                if first:
                                    nc.vector.tensor_copy(out=exs[:],
                                                          in_=ex[:])
                                else:
                                    nc.vector.tensor_add(out=exs[:],
                                                         in0=exs[:],
                                                         in1=ex[:])
                            # partition-reduce exs; reciprocal on DVE; the
                            # ctx evacuates unnormalized (the broadcast
                            # matmuls batch AFTER the qc loop so the PE
                            # never stalls on a fresh reciprocal)
                            pden = psp.tile([1, QCHUNK], f32, tag="acc2",
                                            name="pden", bufs=2)
                            nc.tensor.matmul(pden[:], ones_t[:], exs[:],
                                             start=True, stop=True)
                            rd = ctp.tile([1, QCHUNK], f16, tag="rd",
                                          name="rd", bufs=4)
                            with nc.allow_low_precision(
                                    reason="softmax denom recip f16"):
                                nc.vector.reciprocal(out=rd[:], in_=pden[:])
                            rds.append(rd)
                            cu = ctp.tile([128, QCHUNK], f16,
                                          tag="ctx_t", name="ctx_t", bufs=4)
                            nc.vector.tensor_copy(out=cu[:], in_=pctx[:])
                            ctxu.append(cu)
                        for qc in range(QC):
                            rb2 = psp.tile([128, QCHUNK], f32, tag="acc1",
                                           name="rb2", bufs=2)
                            nc.tensor.matmul(rb2[:], onesr_t[:],
                                             rds[qc][:],
                                             start=True, stop=True)
                            rbs = ctp.tile([128, QCHUNK], f16, tag="rbs",
                                           name="rbs")
                            nc.vector.tensor_copy(out=rbs[:], in_=rb2[:])
                            nc.vector.tensor_mul(out=ctxu[qc][:],
                                                 in0=ctxu[qc][:],
                                                 in1=rbs[:])
                            nc.scalar.dma_start(
                                a2a_in[hl][4 * b + qc, :, :],
                                ctxu[qc][:])
                    # half-A2A for head group hl once both batches done
                    nc.gpsimd.collective_compute(
                        "AllToAll", mybir.AluOpType.bypass,
                        replica_groups=rg,
                        ins=[a2a_in[hl].ap().opt()],
                        outs=[a2a_out[hl].ap().opt()],
                    )

            # ---------- output projection on this core's 512 rows ---------
            # (nested pools reuse the SBUF freed by the QKV/attention pools)
            with tc.tile_pool(name="stageE", bufs=1) as sep, \
                 tc.tile_pool(name="den_sb", bufs=1) as dnp, \
                 tc.tile_pool(name="out_sb", bufs=1) as outp:
                ctx16 = []
                for k in range(HPC):
                    cx = sep.tile([128, NCORES, SHARD], f16,
                                  tag=f"ctx16{k}", name=f"ctx16{k}")
                    nc.sync.dma_start(
                        cx[:],
                        bass.AP(tensor=a2a_out[k], offset=0,
                                ap=[[SHARD, 128], [128 * SHARD, NCORES],
                                    [1, SHARD]]))
                    ctx16.append(cx)

                # output bias broadcast: one-packet row load + ones-matmul
                ob_r = dnp.tile([1, M], f32, tag="ob_r")
                nc.scalar.dma_start(ob_r[:], ob_in.ap()[None, :])
                ob16 = dnp.tile([1, M], f16, tag="ob16")
                nc.vector.tensor_copy(out=ob16[:], in_=ob_r[:])
                ob_t = outp.tile([128, M], f32, tag="ob_t")
                for mc in range(M // MCHUNK):
                    obps = psp.tile([128, MCHUNK], f32, tag="acc2",
                                    name="obps", bufs=2)
                    nc.tensor.matmul(
                        obps[:], onesr_t[:],
                        ob16[0:1, mc * MCHUNK:(mc + 1) * MCHUNK],
                        start=True, stop=True)
                    nc.vector.tensor_copy(
                        out=ob_t[:, mc * MCHUNK:(mc + 1) * MCHUNK],
                        in_=obps[:])

                # even (A2A#0) and odd (A2A#1) head halves contract as
                # SEPARATE accumulation groups: every even-half matmul and
                # its evacuation depends only on A2A#0, so they all run
                # while attention finishes and A2A#1 flies
                o_accs = [outp.tile([128, M], f32, tag=f"o_acc{qt}",
                                    name=f"o_acc{qt}")
                          for qt in range(SHARD // 128)]
                for qt in range(SHARD // 128):
                    for mc in range(M // MCHUNK):
                        po = psp.tile([128, MCHUNK], f32, tag="acc1",
                                      name="po", bufs=2)
                        for i, t in enumerate(range(0, MT, 2)):
                            nc.tensor.matmul(
                                po[:],
                                ctx16[0][:, t // 2,
                                         qt * 128:(qt + 1) * 128],
                                ow_sb[:, t, mc * MCHUNK:(mc + 1) * MCHUNK],
                                start=(i == 0), stop=(i == MT // 2 - 1))
                        nc.vector.tensor_add(
                            out=o_accs[qt][:, mc * MCHUNK:(mc + 1) * MCHUNK],
                            in0=po[:],
                            in1=ob_t[:, mc * MCHUNK:(mc + 1) * MCHUNK])
                for qt in range(SHARD // 128):
                    for mc in range(M // MCHUNK):
                        po = psp.tile([128, MCHUNK], f32,
                                      tag=("ctx", "t3")[mc % 2],
                                      name="po2", bufs=2)
                        for i, t in enumerate(range(1, MT, 2)):
                            nc.tensor.matmul(
                                po[:],
                                ctx16[1][:, t // 2,
                                         qt * 128:(qt + 1) * 128],
                                ow_sb[:, t, mc * MCHUNK:(mc + 1) * MCHUNK],
                                start=(i == 0), stop=(i == MT // 2 - 1))
                        nc.vector.tensor_add(
                            out=o_accs[qt][:, mc * MCHUNK:(mc + 1) * MCHUNK],
                            in0=po[:],
                            in1=o_accs[qt][:, mc * MCHUNK:(mc + 1) * MCHUNK])
                    nc.sync.dma_start(out_ext[qt * 128:(qt + 1) * 128, :],
                                      o_accs[qt][:])

    nc.compile()
    return nc


def _get_program():
    if "nc" not in _NC_CACHE:
        _install_ntff_hook()
        _NC_CACHE["nc"] = _build_program()
    return _NC_CACHE["nc"]


def _prepare_inputs(x, ln_w, ln_b, qkvw, qkvb, ow, ob):
    """Host-side sharding + weight folding. Returns per-core input maps."""
    x = np.asarray(x, dtype=np.float32)
    ln_w = np.asarray(ln_w, dtype=np.float32)
    ln_b = np.asarray(ln_b, dtype=np.float32)
    qkvw = np.asarray(qkvw, dtype=np.float32)
    qkvb = np.asarray(qkvb, dtype=np.float32)
    ow = np.asarray(ow, dtype=np.float32)
    ob = np.asarray(ob, dtype=np.float32)

    xr = np.ascontiguousarray(x.reshape(ROWS, M))
    x16 = xr.astype(np.float16)
    # xng[g, p, j, m] = x[g*256 + j*128 + p, m]
    xng = np.ascontiguousarray(
        x16.reshape(2 * B * NCH, 2, 128, M).transpose(0, 2, 1, 3))
    # xtp[c, p, mt, s] = x[c*512 + s, mt*128 + p]
    xtp = np.ascontiguousarray(
        x16.reshape(B * NCH, CHUNK, MT, 128).transpose(0, 3, 2, 1))
    # fold ln scale/bias into qkv weights/bias
    wp = qkvw * ln_w[None, :]                    # (3M, M)
    bp = qkvw @ ln_b + qkvb                      # (3M,)
    scale = np.float32(1.0 / np.sqrt(D))
    wp[:M] *= scale                              # q rows
    bp[:M] *= scale
    # owtp[p, t, n] = ow[n, t*128 + p]
    owtp = np.ascontiguousarray(
        ow.T.astype(np.float16).reshape(MT, 128, M).transpose(1, 0, 2))

    # causal 0/1 masks in scores^T layout: mask[t, i, j] = (128*t + i) <= j
    ii = np.arange(128)[:, None]
    jj = np.arange(QCHUNK)[None, :]
    mask_const = np.stack(
        [(128 * t + ii <= jj).astype(np.float16) for t in range(4)])
    ident = np.eye(128, dtype=np.float16)

    in_maps = []
    for c in range(NCORES):
        h0 = c * HPC
        rows = []
        for blk in range(2):                     # q rows then k rows
            for hl in range(HPC):
                base = blk * M + (h0 + hl) * D
                rows.append(np.arange(base, base + D))
        qk_rows = np.concatenate(rows)
        v_rows = np.arange(2 * M + h0 * D, 2 * M + (h0 + HPC) * D)
        w_c = np.concatenate([wp[qk_rows], wp[v_rows]], axis=0)   # (768, M)
        w_c16 = w_c.astype(np.float16)
        # wsum must match the fp16 weights actually used on device
        wsum = w_c16.astype(np.float32).sum(axis=1)
        # wtp[p, mt, n] = w_c16[n, mt*128 + p]
        wtp = np.ascontiguousarray(
            w_c16.T.reshape(MT, 128, NW).transpose(1, 0, 2))
        in_maps.append({
            "xng": xng,
            "xtp": xtp,
            "wtp": wtp,
            "wsum_qk": np.ascontiguousarray(wsum[:NQK]),
            "wsum_v": np.ascontiguousarray(wsum[NQK:]),
            "bqk": np.ascontiguousarray(bp[qk_rows]),
            "bv": np.ascontiguousarray(bp[v_rows]),
            "owtp": owtp,
            "ob": ob,
            "mask_const": mask_const,
            "ident": ident,
        })
    return in_maps


def _run(in_maps, trace=False):
    import concourse.bass_utils as bu

    if trace:
        bu.upload_artifacts = lambda tmpdir: "local://" + tmpdir
    nc = _get_program()
    res = bu.run_bass_kernel_spmd(nc, in_maps, list(range(NCORES)),
                                  trace=trace)
    out = np.concatenate(
        [res.results[c]["out_shard"] for c in range(NCORES)], axis=0)
    return out.reshape(B, S, M), res


def kernel(x, ln_w, ln_b, qkvw, qkvb, ow, ob):
    in_maps = _prepare_inputs(x, ln_w, ln_b, qkvw, qkvb, ow, ob)
    out, _ = _run(in_maps, trace=False)
    return out
